# revision 26
# baseline (speedup 1.0000x reference)
"""Trainium2 Bass kernel for the CRA relation module.

Math: the reference computes, per sample,
    phi_x = relu((x@W1+b1)*g1+be1), phi_y likewise,  cat_phi = [phi_x; phi_y]
    A = cat_phi cat_phi^T (symmetric!),  R = [A | A^T] = [A | A]
    W = (cat_phi@W3+b3)@W5a + (R@W4+b4)@W5b + b5
    out = x * W[:196] + y * W[196:]
Because A is symmetric and everything after A is linear into a scalar per
token, the relation pipeline collapses to per-sample matvecs:
    u3 = W3@W5a, u4 = W4@W5b, z = u4[:392]+u4[392:], c0 = b3@W5a+b4@W5b+b5
    s  = u3 + phi_x^T z[:196] + phi_y^T z[196:392]       (768-vector)
    out = x*(phi_x@s + c0) + y*(phi_y@s + c0)
All in bf16 on device (rel err ~5e-3 vs the 2e-2 gate). Data-parallel over
the batch: 16 samples per core on 8 cores.

Layouts: the 768x768 matmuls run feature-major (cin on partitions; host
packs [group, 128, 2*6*392] with [x_a|x_b] 392-col blocks, x blocks then y
blocks). The final per-token reweighting runs token-major (tokens on
partitions) so the per-token weight is a per-partition tensor_scalar
operand: the PE matvec's W row takes a DRAM round trip to turn into a
[196,2] column, and the host supplies a second token-major copy of x,y
([S, 196, 1536] = [x_feat | y_feat]).

Three-phase software pipeline (mains g / reduction+matvec g-2 / final
multiply g-3) keeps the in-order PE, ACT and DVE streams from ever
stalling on the cross-engine tail chain.
"""

import numpy as np
import ml_dtypes
from contextlib import ExitStack

import concourse.bass as bass
import concourse.tile as tile
import concourse.mybir as mybir
from concourse.bass_utils import run_bass_kernel_spmd

F32 = mybir.dt.float32
BF16 = mybir.dt.bfloat16
NPBF = ml_dtypes.bfloat16
ALU = mybir.AluOpType
ACTF = mybir.ActivationFunctionType

B, N, C = 128, 196, 768
NCORES = 8
S = B // NCORES          # 16 samples per core
G = 2                    # samples per weight pass (moving N = 392)
NG = S // G              # 8 groups per core
DT = C // 128            # 6 feature tiles
W2T = 2 * N              # 392
NDVE = 6                 # s-reduction feature tiles on DVE (gpsimd lacks
                         # TensorScalarPtr-with-accum, so all six live here)
N0 = 128                 # token-major chunk split: 196 = 128 + 68
N1 = N - N0


def build_bass(c0: float) -> bass.Bass:
    nc = bass.Bass()
    xy_d = nc.declare_dram_parameter("xy", [NG, 128, 2 * DT * W2T], BF16,
                                     isOutput=False)
    xt_d = nc.declare_dram_parameter("xt", [S, N, 2 * C], BF16, isOutput=False)
    w1_d = nc.declare_dram_parameter("w1", [C, C], BF16, isOutput=False)
    w2_d = nc.declare_dram_parameter("w2", [C, C], BF16, isOutput=False)
    zb_d = nc.declare_dram_parameter("zb", [128, W2T], BF16, isOutput=False)
    u3_d = nc.declare_dram_parameter("u3", [128, DT], F32, isOutput=False)
    b1_d = nc.declare_dram_parameter("b1", [128, DT], F32, isOutput=False)
    b2_d = nc.declare_dram_parameter("b2", [128, DT], F32, isOutput=False)
    out_d = nc.declare_dram_parameter("out", [S, N, C], BF16, isOutput=True)

    with tile.TileContext(nc) as tc, ExitStack() as ctx:
        const = ctx.enter_context(tc.tile_pool(name="const", bufs=1))
        xin = ctx.enter_context(tc.tile_pool(name="xin", bufs=2))
        xtp = ctx.enter_context(tc.tile_pool(name="xtp", bufs=2))
        phip = ctx.enter_context(tc.tile_pool(name="phi", bufs=3))
        sp = ctx.enter_context(tc.tile_pool(name="sp", bufs=3))
        op = ctx.enter_context(tc.tile_pool(name="op", bufs=2))
        ps = ctx.enter_context(tc.tile_pool(name="ps", bufs=2, space="PSUM"))

        # First group's input first: the first psx accumulation needs it plus
        # the w1 k-tiles; everything else can land later.
        YOFF0 = DT * W2T
        xy0 = xin.tile([128, 2 * DT * W2T], BF16, tag="xy", name="xy")
        for c in range(3):
            lo, hi = c * 2 * W2T, (c + 1) * 2 * W2T
            nc.sync.dma_start(out=xy0[:, lo:hi], in_=xy_d[0, :, lo:hi])

        w1_sb, w2_sb = [], []
        for k in range(DT):
            t1 = const.tile([128, C], BF16, tag=f"w1_{k}")
            nc.sync.dma_start(out=t1[:], in_=w1_d[k * 128:(k + 1) * 128, :])
            w1_sb.append(t1)
        b1t = const.tile([128, DT], F32, tag="b1")
        nc.sync.dma_start(out=b1t[:], in_=b1_d[:, :])
        nc.sync.dma_start(out=xy0[:, YOFF0:2 * YOFF0],
                          in_=xy_d[0, :, YOFF0:2 * YOFF0])
        for k in range(DT):
            t2 = const.tile([128, C], BF16, tag=f"w2_{k}")
            nc.sync.dma_start(out=t2[:], in_=w2_d[k * 128:(k + 1) * 128, :])
            w2_sb.append(t2)
        b2t = const.tile([128, DT], F32, tag="b2")
        nc.sync.dma_start(out=b2t[:], in_=b2_d[:, :])
        zb = const.tile([128, W2T], BF16, tag="zb")
        nc.sync.dma_start(out=zb[:], in_=zb_d[:, :])
        u3 = const.tile([128, DT], F32, tag="u3")
        nc.sync.dma_start(out=u3[:], in_=u3_d[:, :])
        # Absorb the bias-tile DMA deps into ACT program order now, so the
        # relu evictions later only ever wait on the PE semaphore (the ISA
        # Activation descriptor holds a single sync-wait).
        ident = const.tile([1, 1], F32, tag="ident")
        nc.vector.memset(ident[:], 1.0)
        warm1 = const.tile([128, 1], F32, tag="warm1")
        warm2 = const.tile([128, 1], F32, tag="warm2")
        nc.scalar.activation(warm1[:], b1t[:, 0:1], ACTF.Copy)
        nc.scalar.activation(warm2[:], b2t[:, 0:1], ACTF.Copy)

        YOFF = DT * W2T

        def emit_mains(g, xyg=None):
            if xyg is None:
                xyg = xin.tile([128, 2 * DT * W2T], BF16, tag="xy", name="xy")
                nc.sync.dma_start(out=xyg[:, 0:YOFF], in_=xy_d[g, :, 0:YOFF])
                nc.sync.dma_start(out=xyg[:, YOFF:2 * YOFF],
                                  in_=xy_d[g, :, YOFF:2 * YOFF])
            # phixy[i][d]: [128, 392] = [phi_x | phi_y] of sample (2g+i), tile d
            phixy = [[phip.tile([128, W2T], BF16, tag=f"phi_{i}_{d}",
                                name=f"phi_{i}_{d}") for d in range(DT)]
                     for i in range(G)]
            t_sbs = [sp.tile([128, DT], F32, tag=f"t_{i}", name=f"t_{i}")
                     for i in range(G)]
            s_sbs = [sp.tile([128, DT], BF16, tag=f"s_{i}", name=f"s_{i}")
                     for i in range(G)]
            for d in range(DT):
                psx = ps.tile([128, W2T], F32, tag="psx", name="psx", bufs=2)
                psy = ps.tile([128, W2T], F32, tag="psy", name="psy", bufs=2)
                for k in range(DT):
                    nc.tensor.matmul(
                        psx[:], w1_sb[k][:, d * 128:(d + 1) * 128],
                        xyg[:, k * W2T:(k + 1) * W2T],
                        start=(k == 0), stop=(k == DT - 1))
                for k in range(DT):
                    nc.tensor.matmul(
                        psy[:], w2_sb[k][:, d * 128:(d + 1) * 128],
                        xyg[:, YOFF + k * W2T:YOFF + (k + 1) * W2T],
                        start=(k == 0), stop=(k == DT - 1))
                for i in range(G):
                    nc.scalar.activation(phixy[i][d][:, 0:N], psx[:, i * N:(i + 1) * N],
                                         ACTF.Relu, bias=b1t[:, d:d + 1])
                    nc.scalar.activation(phixy[i][d][:, N:W2T], psy[:, i * N:(i + 1) * N],
                                         ACTF.Relu, bias=b2t[:, d:d + 1])
                # s-reduction rides along per-d: DVE drains while PE moves on
                for i in range(G):
                    scr = sp.tile([128, W2T], BF16, tag=f"scr_{d}", name="scr")
                    nc.vector.scalar_tensor_tensor(
                        out=scr[:], in0=phixy[i][d][:], scalar=1.0, in1=zb[:],
                        op0=ALU.mult, op1=ALU.mult,
                        accum_out=t_sbs[i][:, d:d + 1])
            for i in range(G):
                nc.vector.tensor_tensor(s_sbs[i][:], t_sbs[i][:], u3[:], ALU.add)
            return xyg, s_sbs, phixy

        def emit_tail_a(g, xyg, s_sbs, phixy):
            """PE matvec, W-row transposed into PSUM columns; xt prefetch."""
            pst = ps.tile([128, 4 * G], F32, tag="pst", name="pst", bufs=2)
            xts = []
            for i in range(G):
                sidx = G * g + i
                xt0 = xtp.tile([N0, 2 * C], BF16, tag=f"xt0_{i}", name=f"xt0_{i}")
                xt1 = xtp.tile([N1, 2 * C], BF16, tag=f"xt1_{i}", name=f"xt1_{i}")
                nc.scalar.dma_start(out=xt0[:], in_=xt_d[sidx, 0:N0, :])
                nc.scalar.dma_start(out=xt1[:], in_=xt_d[sidx, N0:N, :])
                xts.append((xt0, xt1))
                psw = ps.tile([128, W2T], F32, tag="psw", name="psw", bufs=1)
                for d in range(DT):
                    nc.tensor.matmul(psw[0:1, :], s_sbs[i][:, d:d + 1],
                                     phixy[i][d][:],
                                     start=(d == 0), stop=(d == DT - 1))
                wrow = sp.tile([1, W2T], F32, tag="wrow", name="wrow")
                nc.scalar.activation(wrow[:], psw[0:1, :], ACTF.Copy, bias=c0)
                b = 4 * i
                nc.tensor.transpose(pst[:, b + 0:b + 1], wrow[0:1, 0:N0], ident[:])
                nc.tensor.transpose(pst[:, b + 1:b + 2], wrow[0:1, N:N + N0], ident[:])
                nc.tensor.transpose(pst[0:N1, b + 2:b + 3], wrow[0:1, N0:N], ident[:])
                nc.tensor.transpose(pst[0:N1, b + 3:b + 4], wrow[0:1, N + N0:W2T], ident[:])
            return xts, [pst] * G

        def emit_tail_b(g, xts, wcs):
            """out[tok] = x[tok]*(Wx+c0) + y[tok]*(Wy+c0), token-major."""
            for i in range(G):
                (xt0, xt1), pst = xts[i], wcs[i]
                sidx = G * g + i
                b = 4 * i
                for ci, (xt, p0, pn) in enumerate(((xt0, 0, N0), (xt1, N0, N1))):
                    ot = op.tile([pn, C], BF16, tag=f"ot{ci}_{i}",
                                 name=f"ot{ci}_{i}")
                    tmp = op.tile([pn, C], BF16, tag=f"tm{ci}_{i}",
                                  name=f"tm{ci}_{i}")
                    nc.vector.tensor_scalar_mul(
                        tmp[:], xt[:, 0:C], pst[0:pn, b + 2 * ci:b + 2 * ci + 1])
                    nc.vector.scalar_tensor_tensor(
                        out=ot[:], in0=xt[:, C:2 * C],
                        scalar=pst[0:pn, b + 2 * ci + 1:b + 2 * ci + 2],
                        in1=tmp[:], op0=ALU.mult, op1=ALU.add)
                    nc.scalar.dma_start(out=out_d[sidx, p0:p0 + pn, :], in_=ot[:])

        # Three-phase pipeline (lags 1/2): the s-vector is already computed
        # inside mains, so the PE-side tail can follow one group behind.
        mains, tails = {}, {}
        for g in range(NG):
            mains[g] = emit_mains(g, xy0 if g == 0 else None)
            if g - 1 in mains:
                tails[g - 1] = emit_tail_a(g - 1, *mains.pop(g - 1))
            if g - 2 in tails:
                emit_tail_b(g - 2, *tails.pop(g - 2))
        for g in sorted(mains):
            tails[g] = emit_tail_a(g, *mains.pop(g))
        for g in sorted(tails):
            emit_tail_b(g, *tails.pop(g))

    _split_multi_waits(nc)
    return nc


def _split_multi_waits(nc):
    """This walrus build accepts at most ONE sync-wait command per TPB
    instruction; the Tile scheduler happily emits several. Hoist all but the
    last wait of each instruction onto same-engine EventSemaphore ops placed
    immediately before it (engine program order is the within-block
    subsequence, so this preserves semantics)."""
    import json
    data = json.loads(nc.to_json_bytes())
    n = 0
    for fn in data["functions"]:
        for blk in fn["blocks"]:
            out = []
            for inst in blk["instructions"]:
                si = inst.get("sync_info")
                ow = (si or {}).get("on_wait") or []
                if len(ow) > 1:
                    for w in ow[:-1]:
                        n += 1
                        out.append({
                            "name": f"eswait_{n}",
                            "opcode": "EventSemaphore",
                            "engine": inst["engine"],
                            "ins": [],
                            "outs": [],
                            "sync_info": {"on_wait": [w], "on_update": []},
                        })
                    si["on_wait"] = [ow[-1]]
                out.append(inst)
            blk["instructions"] = out
    nc.m = mybir.module_from_json_bytes(json.dumps(data).encode())
    return nc


def prep_host(inputs: dict):
    x = np.ascontiguousarray(np.asarray(inputs["x"], dtype=np.float32))
    y = np.ascontiguousarray(np.asarray(inputs["y"], dtype=np.float32))
    W1 = np.asarray(inputs["W1"], dtype=np.float32)
    W2 = np.asarray(inputs["W2"], dtype=np.float32)
    g1 = np.asarray(inputs["g1"], dtype=np.float32)
    g2 = np.asarray(inputs["g2"], dtype=np.float32)
    b1 = np.asarray(inputs["b1"], dtype=np.float32)
    b2 = np.asarray(inputs["b2"], dtype=np.float32)
    be1 = np.asarray(inputs["be1"], dtype=np.float32)
    be2 = np.asarray(inputs["be2"], dtype=np.float32)
    W3 = np.asarray(inputs["W3"], dtype=np.float32)
    b3 = np.asarray(inputs["b3"], dtype=np.float32)
    W4 = np.asarray(inputs["W4"], dtype=np.float32)
    b4 = np.asarray(inputs["b4"], dtype=np.float32)
    W5 = np.asarray(inputs["W5"], dtype=np.float32)
    b5 = np.asarray(inputs["b5"], dtype=np.float32)

    W1p = np.ascontiguousarray(W1 * g1[None, :]).astype(NPBF)
    W2p = np.ascontiguousarray(W2 * g2[None, :]).astype(NPBF)
    b1p = b1 * g1 + be1
    b2p = b2 * g2 + be2
    W5a, W5b = W5[:C, 0], W5[C:, 0]
    u3 = (W3 @ W5a).astype(np.float32)
    u4 = (W4 @ W5b).astype(np.float32)
    z = (u4[:2 * N] + u4[2 * N:]).astype(np.float32)
    c0 = float(b3 @ W5a + b4 @ W5b + b5[0])

    # [B,N,C] -> per-core groups [M, NG, 128, DT*392] with [x_a|x_b] 392-blocks
    def pack(a):
        at = a.transpose(0, 2, 1).reshape(NCORES, S, DT, 128, N)
        pair = at.reshape(NCORES, NG, G, DT, 128, N)
        gg = np.concatenate([pair[:, :, 0], pair[:, :, 1]], axis=-1)  # [M,NG,DT,128,392]
        return np.ascontiguousarray(
            gg.transpose(0, 1, 3, 2, 4).reshape(NCORES, NG, 128, DT * W2T))

    XY = np.concatenate([pack(x), pack(y)], axis=-1).astype(NPBF)
    # token-major second copy for the final reweighting: [M, S, N, 2C]
    XT = np.concatenate([x, y], axis=-1).reshape(NCORES, S, N, 2 * C).astype(NPBF)
    zb = np.ascontiguousarray(np.broadcast_to(z[None, :], (128, W2T))).astype(NPBF)
    u3t = np.ascontiguousarray(u3.reshape(DT, 128).T)
    b1t = np.ascontiguousarray(b1p.reshape(DT, 128).T)
    b2t = np.ascontiguousarray(b2p.reshape(DT, 128).T)

    in_maps = []
    for cidx in range(NCORES):
        in_maps.append({
            "xy": XY[cidx], "xt": XT[cidx], "w1": W1p, "w2": W2p,
            "zb": zb, "u3": u3t, "b1": b1t, "b2": b2t,
        })
    return in_maps, c0, x, y


def unpack_out(results) -> np.ndarray:
    outs = []
    for cidx in range(NCORES):
        o = np.asarray(results[cidx]["out"]).astype(np.float32)  # [S, N, C]
        outs.append(o)
    return np.ascontiguousarray(np.concatenate(outs, axis=0))


def kernel(**inputs) -> np.ndarray:
    in_maps, c0, _, _ = prep_host(inputs)
    nc = build_bass(c0)
    res = run_bass_kernel_spmd(nc, in_maps, list(range(NCORES)))
    return unpack_out(res.results)


# revision 27
# speedup vs baseline: 1.0995x; 1.0995x over previous
"""Trainium2 Bass kernel for the CRA relation module.

Math: the reference computes, per sample,
    phi_x = relu((x@W1+b1)*g1+be1), phi_y likewise,  cat_phi = [phi_x; phi_y]
    A = cat_phi cat_phi^T (symmetric!),  R = [A | A^T] = [A | A]
    W = (cat_phi@W3+b3)@W5a + (R@W4+b4)@W5b + b5
    out = x * W[:196] + y * W[196:]
Because A is symmetric and everything after A is linear into a scalar per
token, the relation pipeline collapses to per-sample matvecs:
    u3 = W3@W5a, u4 = W4@W5b, z = u4[:392]+u4[392:], c0 = b3@W5a+b4@W5b+b5
    s  = u3 + phi_x^T z[:196] + phi_y^T z[196:392]       (768-vector)
    out = x*(phi_x@s + c0) + y*(phi_y@s + c0)
All in bf16 on device (rel err ~5e-3 vs the 2e-2 gate). Data-parallel over
the batch: 16 samples per core on 8 cores.

Layouts: the 768x768 matmuls run feature-major (cin on partitions; host
packs [group, 128, 2*6*392] with [x_a|x_b] 392-col blocks, x blocks then y
blocks). The final per-token reweighting runs token-major (tokens on
partitions) so the per-token weight is a per-partition tensor_scalar
operand: the PE matvec's W row takes a DRAM round trip to turn into a
[196,2] column, and the host supplies a second token-major copy of x,y
([S, 196, 1536] = [x_feat | y_feat]).

Three-phase software pipeline (mains g / reduction+matvec g-2 / final
multiply g-3) keeps the in-order PE, ACT and DVE streams from ever
stalling on the cross-engine tail chain.
"""

import numpy as np
import ml_dtypes
from contextlib import ExitStack

import concourse.bass as bass
import concourse.tile as tile
import concourse.mybir as mybir
from concourse.bass_utils import run_bass_kernel_spmd

F32 = mybir.dt.float32
BF16 = mybir.dt.bfloat16
NPBF = ml_dtypes.bfloat16
ALU = mybir.AluOpType
ACTF = mybir.ActivationFunctionType

B, N, C = 128, 196, 768
NCORES = 8
S = B // NCORES          # 16 samples per core
G = 2                    # samples per weight pass (moving N = 392)
NG = S // G              # 8 groups per core
DT = C // 128            # 6 feature tiles
W2T = 2 * N              # 392
NDVE = 6                 # s-reduction feature tiles on DVE (gpsimd lacks
                         # TensorScalarPtr-with-accum, so all six live here)
N0 = 128                 # token-major chunk split: 196 = 128 + 68
N1 = N - N0


def build_bass(c0: float) -> bass.Bass:
    nc = bass.Bass()
    xy_d = nc.declare_dram_parameter("xy", [NG, 128, 2 * DT * W2T], BF16,
                                     isOutput=False)
    xt_d = nc.declare_dram_parameter("xt", [S, N, 2 * C], BF16, isOutput=False)
    w1_d = nc.declare_dram_parameter("w1", [C, C], BF16, isOutput=False)
    w2_d = nc.declare_dram_parameter("w2", [C, C], BF16, isOutput=False)
    zb_d = nc.declare_dram_parameter("zb", [128, W2T], BF16, isOutput=False)
    u3_d = nc.declare_dram_parameter("u3", [128, DT], F32, isOutput=False)
    b1_d = nc.declare_dram_parameter("b1", [128, DT], F32, isOutput=False)
    b2_d = nc.declare_dram_parameter("b2", [128, DT], F32, isOutput=False)
    out_d = nc.declare_dram_parameter("out", [S, N, C], BF16, isOutput=True)

    with tile.TileContext(nc) as tc, ExitStack() as ctx:
        const = ctx.enter_context(tc.tile_pool(name="const", bufs=1))
        xin = ctx.enter_context(tc.tile_pool(name="xin", bufs=2))
        xtp = ctx.enter_context(tc.tile_pool(name="xtp", bufs=2))
        phip = ctx.enter_context(tc.tile_pool(name="phi", bufs=3))
        sp = ctx.enter_context(tc.tile_pool(name="sp", bufs=3))
        op = ctx.enter_context(tc.tile_pool(name="op", bufs=2))
        ps = ctx.enter_context(tc.tile_pool(name="ps", bufs=2, space="PSUM"))

        # First group's input first: the first psx accumulation needs it plus
        # the w1 k-tiles; everything else can land later.
        YOFF0 = DT * W2T
        xy0 = xin.tile([128, 2 * DT * W2T], BF16, tag="xy", name="xy")
        for c in range(3):
            lo, hi = c * 2 * W2T, (c + 1) * 2 * W2T
            nc.sync.dma_start(out=xy0[:, lo:hi], in_=xy_d[0, :, lo:hi])

        w1_sb, w2_sb = [], []
        for k in range(DT):
            t1 = const.tile([128, C], BF16, tag=f"w1_{k}")
            nc.sync.dma_start(out=t1[:], in_=w1_d[k * 128:(k + 1) * 128, :])
            w1_sb.append(t1)
        b1t = const.tile([128, DT], F32, tag="b1")
        nc.sync.dma_start(out=b1t[:], in_=b1_d[:, :])
        nc.sync.dma_start(out=xy0[:, YOFF0:2 * YOFF0],
                          in_=xy_d[0, :, YOFF0:2 * YOFF0])
        for k in range(DT):
            t2 = const.tile([128, C], BF16, tag=f"w2_{k}")
            nc.sync.dma_start(out=t2[:], in_=w2_d[k * 128:(k + 1) * 128, :])
            w2_sb.append(t2)
        b2t = const.tile([128, DT], F32, tag="b2")
        nc.sync.dma_start(out=b2t[:], in_=b2_d[:, :])
        zb = const.tile([128, W2T], BF16, tag="zb")
        nc.sync.dma_start(out=zb[:], in_=zb_d[:, :])
        u3 = const.tile([128, DT], F32, tag="u3")
        nc.sync.dma_start(out=u3[:], in_=u3_d[:, :])
        # Absorb the bias-tile DMA deps into ACT program order now, so the
        # relu evictions later only ever wait on the PE semaphore (the ISA
        # Activation descriptor holds a single sync-wait).
        ident = const.tile([1, 1], F32, tag="ident")
        nc.vector.memset(ident[:], 1.0)
        warm1 = const.tile([128, 1], F32, tag="warm1")
        warm2 = const.tile([128, 1], F32, tag="warm2")
        nc.scalar.activation(warm1[:], b1t[:, 0:1], ACTF.Copy)
        nc.scalar.activation(warm2[:], b2t[:, 0:1], ACTF.Copy)

        YOFF = DT * W2T

        def emit_mains(g, xyg=None):
            if xyg is None:
                xyg = xin.tile([128, 2 * DT * W2T], BF16, tag="xy", name="xy")
                nc.sync.dma_start(out=xyg[:, 0:YOFF], in_=xy_d[g, :, 0:YOFF])
                nc.sync.dma_start(out=xyg[:, YOFF:2 * YOFF],
                                  in_=xy_d[g, :, YOFF:2 * YOFF])
            # phixy[i][d]: [128, 392] = [phi_x | phi_y] of sample (2g+i), tile d
            phixy = [[phip.tile([128, W2T], BF16, tag=f"phi_{i}_{d}",
                                name=f"phi_{i}_{d}") for d in range(DT)]
                     for i in range(G)]
            t_sbs = [sp.tile([128, DT], F32, tag=f"t_{i}", name=f"t_{i}")
                     for i in range(G)]
            s_sbs = [sp.tile([128, DT], BF16, tag=f"s_{i}", name=f"s_{i}")
                     for i in range(G)]
            for d in range(DT):
                psx = ps.tile([128, W2T], F32, tag="psx", name="psx", bufs=2)
                psy = ps.tile([128, W2T], F32, tag="psy", name="psy", bufs=2)
                for k in range(DT):
                    nc.tensor.matmul(
                        psx[:], w1_sb[k][:, d * 128:(d + 1) * 128],
                        xyg[:, k * W2T:(k + 1) * W2T],
                        start=(k == 0), stop=(k == DT - 1))
                for k in range(DT):
                    nc.tensor.matmul(
                        psy[:], w2_sb[k][:, d * 128:(d + 1) * 128],
                        xyg[:, YOFF + k * W2T:YOFF + (k + 1) * W2T],
                        start=(k == 0), stop=(k == DT - 1))
                for i in range(G):
                    nc.scalar.activation(phixy[i][d][:, 0:N], psx[:, i * N:(i + 1) * N],
                                         ACTF.Relu, bias=b1t[:, d:d + 1])
                    nc.scalar.activation(phixy[i][d][:, N:W2T], psy[:, i * N:(i + 1) * N],
                                         ACTF.Relu, bias=b2t[:, d:d + 1])
                # s-reduction rides along per-d: DVE drains while PE moves on
                for i in range(G):
                    scr = sp.tile([128, W2T], BF16, tag=f"scr_{d}", name="scr")
                    nc.vector.scalar_tensor_tensor(
                        out=scr[:], in0=phixy[i][d][:], scalar=1.0, in1=zb[:],
                        op0=ALU.mult, op1=ALU.mult,
                        accum_out=t_sbs[i][:, d:d + 1])
            for i in range(G):
                nc.vector.tensor_tensor(s_sbs[i][:], t_sbs[i][:], u3[:], ALU.add)
            return xyg, s_sbs, phixy

        def emit_tail_a(g, xyg, s_sbs, phixy):
            """PE matvec, W-row transposed into PSUM columns; xt prefetch."""
            pst = ps.tile([128, 4 * G], F32, tag="pst", name="pst", bufs=2)
            xts = []
            for i in range(G):
                sidx = G * g + i
                xt0 = xtp.tile([N0, 2 * C], BF16, tag=f"xt0_{i}", name=f"xt0_{i}")
                xt1 = xtp.tile([N1, 2 * C], BF16, tag=f"xt1_{i}", name=f"xt1_{i}")
                nc.sync.dma_start(out=xt0[:], in_=xt_d[sidx, 0:N0, :])
                nc.sync.dma_start(out=xt1[:], in_=xt_d[sidx, N0:N, :])
                xts.append((xt0, xt1))
                psw = ps.tile([128, W2T], F32, tag="psw", name="psw", bufs=1)
                for d in range(DT):
                    nc.tensor.matmul(psw[0:1, :], s_sbs[i][:, d:d + 1],
                                     phixy[i][d][:],
                                     start=(d == 0), stop=(d == DT - 1))
                wrow = sp.tile([1, W2T], F32, tag="wrow", name="wrow")
                nc.scalar.activation(wrow[:], psw[0:1, :], ACTF.Copy, bias=c0)
                b = 4 * i
                nc.tensor.transpose(pst[:, b + 0:b + 1], wrow[0:1, 0:N0], ident[:])
                nc.tensor.transpose(pst[:, b + 1:b + 2], wrow[0:1, N:N + N0], ident[:])
                nc.tensor.transpose(pst[0:N1, b + 2:b + 3], wrow[0:1, N0:N], ident[:])
                nc.tensor.transpose(pst[0:N1, b + 3:b + 4], wrow[0:1, N + N0:W2T], ident[:])
            return xts, [pst] * G

        def emit_tail_b(g, xts, wcs):
            """out[tok] = x[tok]*(Wx+c0) + y[tok]*(Wy+c0), token-major."""
            for i in range(G):
                (xt0, xt1), pst = xts[i], wcs[i]
                sidx = G * g + i
                b = 4 * i
                for ci, (xt, p0, pn) in enumerate(((xt0, 0, N0), (xt1, N0, N1))):
                    ot = op.tile([pn, C], BF16, tag=f"ot{ci}_{i}",
                                 name=f"ot{ci}_{i}")
                    tmp = op.tile([pn, C], BF16, tag=f"tm{ci}_{i}",
                                  name=f"tm{ci}_{i}")
                    nc.vector.tensor_scalar_mul(
                        tmp[:], xt[:, 0:C], pst[0:pn, b + 2 * ci:b + 2 * ci + 1])
                    nc.vector.scalar_tensor_tensor(
                        out=ot[:], in0=xt[:, C:2 * C],
                        scalar=pst[0:pn, b + 2 * ci + 1:b + 2 * ci + 2],
                        in1=tmp[:], op0=ALU.mult, op1=ALU.add)
                    nc.sync.dma_start(out=out_d[sidx, p0:p0 + pn, :], in_=ot[:])

        # Three-phase pipeline (lags 1/2): the s-vector is already computed
        # inside mains, so the PE-side tail can follow one group behind.
        mains, tails = {}, {}
        for g in range(NG):
            mains[g] = emit_mains(g, xy0 if g == 0 else None)
            if g - 1 in mains:
                tails[g - 1] = emit_tail_a(g - 1, *mains.pop(g - 1))
            if g - 2 in tails:
                emit_tail_b(g - 2, *tails.pop(g - 2))
        for g in sorted(mains):
            tails[g] = emit_tail_a(g, *mains.pop(g))
        for g in sorted(tails):
            emit_tail_b(g, *tails.pop(g))

    _split_multi_waits(nc)
    return nc


def _split_multi_waits(nc):
    """This walrus build accepts at most ONE sync-wait command per TPB
    instruction; the Tile scheduler happily emits several. Hoist all but the
    last wait of each instruction onto same-engine EventSemaphore ops placed
    immediately before it (engine program order is the within-block
    subsequence, so this preserves semantics)."""
    import json
    data = json.loads(nc.to_json_bytes())
    n = 0
    for fn in data["functions"]:
        for blk in fn["blocks"]:
            out = []
            for inst in blk["instructions"]:
                si = inst.get("sync_info")
                ow = (si or {}).get("on_wait") or []
                if len(ow) > 1:
                    for w in ow[:-1]:
                        n += 1
                        out.append({
                            "name": f"eswait_{n}",
                            "opcode": "EventSemaphore",
                            "engine": inst["engine"],
                            "ins": [],
                            "outs": [],
                            "sync_info": {"on_wait": [w], "on_update": []},
                        })
                    si["on_wait"] = [ow[-1]]
                out.append(inst)
            blk["instructions"] = out
    nc.m = mybir.module_from_json_bytes(json.dumps(data).encode())
    return nc


def prep_host(inputs: dict):
    x = np.ascontiguousarray(np.asarray(inputs["x"], dtype=np.float32))
    y = np.ascontiguousarray(np.asarray(inputs["y"], dtype=np.float32))
    W1 = np.asarray(inputs["W1"], dtype=np.float32)
    W2 = np.asarray(inputs["W2"], dtype=np.float32)
    g1 = np.asarray(inputs["g1"], dtype=np.float32)
    g2 = np.asarray(inputs["g2"], dtype=np.float32)
    b1 = np.asarray(inputs["b1"], dtype=np.float32)
    b2 = np.asarray(inputs["b2"], dtype=np.float32)
    be1 = np.asarray(inputs["be1"], dtype=np.float32)
    be2 = np.asarray(inputs["be2"], dtype=np.float32)
    W3 = np.asarray(inputs["W3"], dtype=np.float32)
    b3 = np.asarray(inputs["b3"], dtype=np.float32)
    W4 = np.asarray(inputs["W4"], dtype=np.float32)
    b4 = np.asarray(inputs["b4"], dtype=np.float32)
    W5 = np.asarray(inputs["W5"], dtype=np.float32)
    b5 = np.asarray(inputs["b5"], dtype=np.float32)

    W1p = np.ascontiguousarray(W1 * g1[None, :]).astype(NPBF)
    W2p = np.ascontiguousarray(W2 * g2[None, :]).astype(NPBF)
    b1p = b1 * g1 + be1
    b2p = b2 * g2 + be2
    W5a, W5b = W5[:C, 0], W5[C:, 0]
    u3 = (W3 @ W5a).astype(np.float32)
    u4 = (W4 @ W5b).astype(np.float32)
    z = (u4[:2 * N] + u4[2 * N:]).astype(np.float32)
    c0 = float(b3 @ W5a + b4 @ W5b + b5[0])

    # [B,N,C] -> per-core groups [M, NG, 128, DT*392] with [x_a|x_b] 392-blocks
    def pack(a):
        at = a.transpose(0, 2, 1).reshape(NCORES, S, DT, 128, N)
        pair = at.reshape(NCORES, NG, G, DT, 128, N)
        gg = np.concatenate([pair[:, :, 0], pair[:, :, 1]], axis=-1)  # [M,NG,DT,128,392]
        return np.ascontiguousarray(
            gg.transpose(0, 1, 3, 2, 4).reshape(NCORES, NG, 128, DT * W2T))

    XY = np.concatenate([pack(x), pack(y)], axis=-1).astype(NPBF)
    # token-major second copy for the final reweighting: [M, S, N, 2C]
    XT = np.concatenate([x, y], axis=-1).reshape(NCORES, S, N, 2 * C).astype(NPBF)
    zb = np.ascontiguousarray(np.broadcast_to(z[None, :], (128, W2T))).astype(NPBF)
    u3t = np.ascontiguousarray(u3.reshape(DT, 128).T)
    b1t = np.ascontiguousarray(b1p.reshape(DT, 128).T)
    b2t = np.ascontiguousarray(b2p.reshape(DT, 128).T)

    in_maps = []
    for cidx in range(NCORES):
        in_maps.append({
            "xy": XY[cidx], "xt": XT[cidx], "w1": W1p, "w2": W2p,
            "zb": zb, "u3": u3t, "b1": b1t, "b2": b2t,
        })
    return in_maps, c0, x, y


def unpack_out(results) -> np.ndarray:
    outs = []
    for cidx in range(NCORES):
        o = np.asarray(results[cidx]["out"]).astype(np.float32)  # [S, N, C]
        outs.append(o)
    return np.ascontiguousarray(np.concatenate(outs, axis=0))


def kernel(**inputs) -> np.ndarray:
    in_maps, c0, _, _ = prep_host(inputs)
    nc = build_bass(c0)
    res = run_bass_kernel_spmd(nc, in_maps, list(range(NCORES)))
    return unpack_out(res.results)
